# revision 2
# baseline (speedup 1.0000x reference)
"""Trainium2 Bass kernel for nn_GCN2 — v3: dense edge streams + PE segment reduce.

Math (matching the reference):
    T0 = X;  T1 = L X;  T2 = 2 L T1 - T0
    h1 = X (W1a - W1c) + L (X W1b) + 2 L (L (X W1c))
    x_l = relu(S h_l + b_l),  S = D^-1/2 (A + I) D^-1/2,  h_l = x_{l-1} W_l
    out = softmax(((x1+x2+x3)/3 pooled-by-graph-mean) Wout + bout)

SpMM strategy (v3): edges are dest-sorted into dense 128-edge columns with a
canonical cross-core slot grid (per (chunk, tile, 32-lane bin, stream) the
capacity is the max edge count over the 8 cores, ~7% padding).  dma_gather
fetches each column's source rows (idx i -> partition i%128, slot i//128),
then the TensorEngine reduces columns into per-dest-tile PSUM accumulators
using host-precomputed sparse selector blocks (lhsT [128 edges, 32 dests],
edge values folded in).  GpSimd processes ~true-edge-count indices (the
SWDGE descriptor-gen rate of ~8ns/idx is the hard floor), and the reduce
runs on the otherwise-idle PE instead of the vector engine.

Self-loops for conv2/conv3 are computed locally (dest and table share the
permA ordering), removing 2*6250 edges/core from the gathers.
"""

import numpy as np

import concourse.bass as bass
import concourse.bacc as bacc
import concourse.mybir as mybir
import concourse.tile as tile
from concourse.bass_utils import run_bass_kernel_spmd

# ---- problem geometry ----
N = 50000
DIN = 128
F = 64
NG = 512
DOUT = 10

NCORES = 8
P = 128
TPC = 49
NPC = TPC * P             # 6272
NTOT = NCORES * NPC       # 50176
VIEW = 32768
HI_BASE = NTOT - VIEW     # 17408

WIN = 32                  # selector dest-window width (psum bin)
NBIN = P // WIN           # 4 bins per tile
TPCH = 3                  # tiles per chunk
NCHUNK = (TPC + TPCH - 1) // TPCH

F32 = mybir.dt.float32
BF16 = mybir.dt.bfloat16
I16 = mybir.dt.int16

LAST_EXEC_TIME_NS = None


# ---------------- host-side preprocessing ----------------

def _perm_from_indeg(indeg):
    order = np.argsort(-indeg, kind="stable").astype(np.int64)
    perm = np.full(NTOT, -1, np.int64)
    for c in range(NCORES):
        own = order[c::NCORES]
        perm[c * NPC: c * NPC + own.size] = own
    inv = np.full(N, -1, np.int64)
    mask = perm >= 0
    inv[perm[mask]] = np.nonzero(mask)[0]
    return perm, inv


def _wrap16(flat_i32):
    n = flat_i32.size
    assert n % 16 == 0
    a = flat_i32.reshape(n // 16, 16).T.astype(np.int16)
    return np.ascontiguousarray(np.tile(a, (8, 1)))


def _build_stream_stage(drow_g, srow, vals, sel_np_dt=np.float32):
    """Canonical-grid dense edge streams shared by all 8 cores.

    Returns (idx_arrs[8], sel_arrs[8], chunks) where chunks is the shared
    compile-time metadata:
      chunks[ci] = {nlo, nhi, npieces, events}
      events: ("p", stream, col, tile_local, bin, start, stop) | ("e", tile_local)
    """
    if vals is None:
        vals = np.ones(drow_g.size, np.float32)

    # per-core edge lists (dest-sorted)
    cores = []
    core_of = drow_g // NPC
    for c in range(NCORES):
        m = core_of == c
        d = (drow_g[m] - c * NPC).astype(np.int64)
        s = srow[m].astype(np.int64)
        v = vals[m].astype(np.float32)
        o = np.argsort(d, kind="stable")
        d, s, v = d[o], s[o], v[o]
        # stream: 0=lo view, 1=hi view; flex balanced per (tile,bin) group
        can_lo = s < VIEW
        can_hi = s >= HI_BASE
        st = np.where(can_lo & ~can_hi, 0, np.where(can_hi & ~can_lo, 1, -1))
        grp = (d // P) * NBIN + (d % P) // WIN     # global group id (t*4+b)
        # balance flex within each group
        for g in np.unique(grp[st < 0]):
            gm = grp == g
            nlo = int(((st == 0) & gm).sum())
            nhi = int(((st == 1) & gm).sum())
            fx = np.nonzero(gm & (st < 0))[0]
            want_lo = max(0, min(fx.size, (nhi - nlo + fx.size + 1) // 2))
            st[fx[:want_lo]] = 0
            st[fx[want_lo:]] = 1
        cores.append((d, s, v, st, grp))

    NGRP = TPC * NBIN
    # capacity per (group, stream) = max over cores
    cap = np.zeros((NGRP, 2), np.int64)
    for (d, s, v, st, grp) in cores:
        for stv in (0, 1):
            cnt = np.bincount(grp[st == stv], minlength=NGRP)
            cap[:, stv] = np.maximum(cap[:, stv], cnt)
    # ensure every group has at least one slot total (for psum init)
    both0 = (cap[:, 0] + cap[:, 1]) == 0
    cap[both0, 0] = 1

    # slot bases per (chunk, stream)
    chunks = []
    piece_index = {}        # (ci, st, col, g) -> pid (in emission order)
    slot_base = np.zeros((NGRP, 2), np.int64)
    chunk_cols = np.zeros((NCHUNK, 2), np.int64)
    for ci in range(NCHUNK):
        t0, t1 = ci * TPCH, min((ci + 1) * TPCH, TPC)
        ginc = np.arange(t0 * NBIN, t1 * NBIN)
        ncols = []
        for stv in (0, 1):
            c_ = cap[ginc, stv]
            b_ = np.zeros(c_.size, np.int64)
            np.cumsum(c_[:-1], out=b_[1:])
            slot_base[ginc, stv] = b_
            S = int(c_.sum())
            ncols.append((S + P - 1) // P)
        chunk_cols[ci] = ncols

        # canonical pieces: for each (stream, col): overlapped groups
        pieces_by_tile = {t: {0: [], 1: []} for t in range(t0, t1)}
        for stv in (0, 1):
            for g in ginc:
                c0 = int(slot_base[g, stv])
                c1 = c0 + int(cap[g, stv])
                if c1 == c0:
                    continue
                t = int(g) // NBIN
                b = int(g) % NBIN
                for col in range(c0 // P, (c1 - 1) // P + 1):
                    pieces_by_tile[t][stv].append((col, b))
        # emission: per tile: lo pieces, hi pieces, epilogue
        events = []
        order = []
        for t in range(t0, t1):
            for stv in (0, 1):
                for (col, b) in pieces_by_tile[t][stv]:
                    order.append((stv, col, t, b))
            order.append(("e", t))
        first = {}
        last = {}
        for gpi, it in enumerate(order):
            if it[0] == "e":
                continue
            kk = (it[2], it[3])
            if kk not in first:
                first[kk] = gpi
            last[kk] = gpi
        npieces = 0
        for gpi, it in enumerate(order):
            if it[0] == "e":
                events.append(("e", it[1] - t0))
                continue
            stv, col, t, b = it
            kk = (t, b)
            piece_index[(ci, stv, col, t * NBIN + b)] = npieces
            events.append(("p", stv, col, t - t0, b,
                           first[kk] == gpi, last[kk] == gpi))
            npieces += 1
        chunks.append({"nlo": int(ncols[0]), "nhi": int(ncols[1]),
                       "npieces": npieces, "events": events})

    # prefix offsets for idx/sel arrays
    tot_cols = int(chunk_cols.sum())
    tot_pieces = sum(ch["npieces"] for ch in chunks)

    idx_arrs = []
    sel_arrs = []
    for (d, s, v, st, grp) in cores:
        # slot of each edge: base + rank within (group, stream)
        slot = np.zeros(d.size, np.int64)
        colg = np.zeros(d.size, np.int64)   # global column id in idx layout
        laneg = np.zeros(d.size, np.int64)
        pid = np.zeros(d.size, np.int64)
        ci_e = d // (P * TPCH)
        idx_flat = np.zeros(tot_cols * P, np.int32)
        sel = np.zeros((P, tot_pieces * WIN), sel_np_dt)
        col_off = 0
        piece_off = 0
        for ci in range(NCHUNK):
            for stv in (0, 1):
                em = (ci_e == ci) & (st == stv)
                ge = grp[em]
                # rank within group (edges are dest-sorted => grouped order)
                r = np.zeros(ge.size, np.int64)
                if ge.size:
                    gb = np.zeros(NGRP + 1, np.int64)
                    np.cumsum(np.bincount(ge, minlength=NGRP), out=gb[1:])
                    r = np.arange(ge.size) - gb[ge]
                sl = slot_base[ge, stv] + r
                col_l = sl // P
                lane = sl % P
                gcol = col_off + col_l
                idx_flat[gcol * P + lane] = s[em] - (HI_BASE if stv else 0)
                # piece id per edge
                pe = np.fromiter(
                    (piece_index[(ci, stv, int(cc), int(gg))]
                     for cc, gg in zip(col_l, ge)),
                    np.int64, ge.size) if ge.size else np.zeros(0, np.int64)
                dl = (d[em] % P) - (ge % NBIN) * WIN
                sel[lane, (piece_off + pe) * WIN + dl] = v[em]
                col_off += int(chunk_cols[ci, stv])
            piece_off += chunks[ci]["npieces"]
        # wrap16 per column-block is global: the gather consumes each call's
        # block [cols*128] linearly => wrap the whole flat array per chunk
        # slices. We wrap per (chunk, stream) block.
        parts = []
        off = 0
        for ci in range(NCHUNK):
            for stv in (0, 1):
                nc_ = int(chunk_cols[ci, stv])
                if nc_ == 0:
                    continue
                blk = idx_flat[off * P:(off + nc_) * P]
                parts.append(_wrap16(blk))
                off += nc_
        idx_arrs.append(np.ascontiguousarray(
            np.concatenate(parts, axis=1) if parts
            else np.zeros((P, 8), np.int16)))
        sel_arrs.append(np.ascontiguousarray(sel))
    return idx_arrs, sel_arrs, chunks


def _prep(X, L_indices, L_values, batch, W1, W2, W3, Wout, b1, b2, b3, bout):
    Ls, Ld = L_indices[1].astype(np.int64), L_indices[0].astype(np.int64)
    Arow, Acol = L_indices[0].astype(np.int64), L_indices[1].astype(np.int64)

    deg = np.bincount(Acol, minlength=N).astype(np.float64) + 1.0
    dis = (1.0 / np.sqrt(deg)).astype(np.float32)

    indeg_L = np.bincount(Ld, minlength=N)
    indeg_A = np.bincount(Acol, minlength=N) + 1
    permL, invL = _perm_from_indeg(indeg_L)
    permA, invA = _perm_from_indeg(indeg_A)

    import ml_dtypes
    idxL, selL, chunksL = _build_stream_stage(
        invL[Ld], invL[Ls], np.asarray(L_values, np.float32),
        sel_np_dt=ml_dtypes.bfloat16)
    selL2 = [s.astype(np.float32) for s in selL]

    sl = np.arange(N, dtype=np.int64)
    d1 = invA[np.concatenate([Acol, sl])]
    s1 = invL[np.concatenate([Arow, sl])]
    idxA1, selA1, chunksA1 = _build_stream_stage(d1, s1, None)

    idxA23, selA23, chunksA23 = _build_stream_stage(
        invA[Acol], invA[Arow], None)

    Xp = np.zeros((NTOT, DIN), np.float32)
    mask = permL >= 0
    Xp[mask] = np.asarray(X, np.float32)[permL[mask]]
    XT = np.ascontiguousarray(Xp.T)

    disL = np.zeros((NTOT, 1), np.float32)
    disL[mask, 0] = dis[permL[mask]]
    maskA = permA >= 0
    disA = np.zeros((NTOT, 1), np.float32)
    disA[maskA, 0] = dis[permA[maskA]]
    batchA = np.full((NTOT, 1), -1.0, np.float32)
    batchA[maskA, 0] = np.asarray(batch, np.float32)[permA[maskA]]

    W1 = np.asarray(W1, np.float32)
    W1a, W1b, W1c = W1[:DIN], W1[DIN:2 * DIN], W1[2 * DIN:]
    W1ac = np.ascontiguousarray(W1a - W1c)
    W1bc = np.ascontiguousarray(np.concatenate([W1b, W1c], axis=1))

    counts = np.bincount(np.asarray(batch, np.int64),
                         minlength=NG).astype(np.float64)
    inv3n = (1.0 / (3.0 * np.maximum(counts, 1.0))).astype(np.float32)[:, None]
    grid = np.broadcast_to(np.arange(NG, dtype=np.float32)[None, :],
                           (P, NG)).copy()

    import ml_dtypes as _mld
    rep = dict(
        XP=np.ascontiguousarray(Xp.astype(_mld.bfloat16)),
        W1ac=W1ac, W1bc=W1bc,
        W2=np.asarray(W2, np.float32), W3=np.asarray(W3, np.float32),
        Wout=np.asarray(Wout, np.float32),
        b1r=np.tile(np.asarray(b1, np.float32)[None, :], (P, 1)),
        b2r=np.tile(np.asarray(b2, np.float32)[None, :], (P, 1)),
        b3r=np.tile(np.asarray(b3, np.float32)[None, :], (P, 1)),
        boutr=np.tile(np.asarray(bout, np.float32)[None, :], (P, 1)),
        grid=grid, inv3n=inv3n,
        ident_in=np.eye(P, dtype=np.float32),
        dummy_tab=np.zeros((P, F), np.float32),
        dummy_idx=np.zeros((P, 8), np.int16),
    )

    in_maps = []
    for c in range(NCORES):
        r0 = c * NPC
        m = dict(rep)
        m["XTOWN"] = np.ascontiguousarray(XT[:, r0:r0 + NPC])
        m["disL"] = disL[r0:r0 + NPC].copy()
        m["disA"] = disA[r0:r0 + NPC].copy()
        m["batchA"] = batchA[r0:r0 + NPC].copy()
        m["IDXL"] = idxL[c]
        m["SELL"] = selL[c]
        m["SELL2"] = selL2[c]
        m["IDXA1"] = idxA1[c]
        m["SELA1"] = selA1[c]
        m["IDXA23"] = idxA23[c]
        m["SELA23"] = selA23[c]
        in_maps.append(m)

    meta = {"chunksL": chunksL, "chunksA1": chunksA1, "chunksA23": chunksA23,
            "w_IDXL": idxL[0].shape[1], "w_SELL": selL[0].shape[1],
            "w_SELL2": selL2[0].shape[1],
            "w_IDXA1": idxA1[0].shape[1], "w_SELA1": selA1[0].shape[1],
            "w_IDXA23": idxA23[0].shape[1], "w_SELA23": selA23[0].shape[1]}
    return in_maps, meta


# ---------------- device program ----------------

def _build_program(meta):
    nc = bacc.Bacc("TRN2", target_bir_lowering=False, debug=False,
                   num_devices=NCORES)

    def din(name, shape, dt=F32):
        return nc.dram_tensor(name, shape, dt, kind="ExternalInput").ap()

    chunksL = meta["chunksL"]
    chunksA1 = meta["chunksA1"]
    chunksA23 = meta["chunksA23"]

    XP = din("XP", [NTOT, DIN], BF16)
    XTOWN = din("XTOWN", [P, NPC])
    W1ac = din("W1ac", [DIN, F])
    W1bc = din("W1bc", [DIN, DIN])
    W2 = din("W2", [F, F])
    W3 = din("W3", [F, F])
    Wout = din("Wout", [F, DOUT])
    b1r = din("b1r", [P, F])
    b2r = din("b2r", [P, F])
    b3r = din("b3r", [P, F])
    boutr = din("boutr", [P, DOUT])
    grid = din("grid", [P, NG])
    ident_in = din("ident_in", [P, P])
    dummy_tab = din("dummy_tab", [P, F])
    dummy_idx = din("dummy_idx", [P, 8], I16)
    inv3n = din("inv3n", [NG, 1])
    disL_d = din("disL", [NPC, 1])
    disA_d = din("disA", [NPC, 1])
    batchA_d = din("batchA", [NPC, 1])
    IDXL = din("IDXL", [P, meta["w_IDXL"]], I16)
    SELL = din("SELL", [P, meta["w_SELL"]], BF16)
    SELL2 = din("SELL2", [P, meta["w_SELL2"]])
    IDXA1 = din("IDXA1", [P, meta["w_IDXA1"]], I16)
    SELA1 = din("SELA1", [P, meta["w_SELA1"]])
    IDXA23 = din("IDXA23", [P, meta["w_IDXA23"]], I16)
    SELA23 = din("SELA23", [P, meta["w_SELA23"]])

    OUT = nc.dram_tensor("out", [NG, DOUT], F32, kind="ExternalOutput").ap()

    with tile.TileContext(nc) as tc:
        with (
            tc.tile_pool(name="dram", bufs=1, space="DRAM") as dr,
            tc.tile_pool(name="sbuf", bufs=1) as sb,
            tc.tile_pool(name="psum", bufs=1, space="PSUM") as ps,
        ):
            lc_local = dr.tile([NPC, F], F32, name="lc_local")
            lc_table = dr.tile([NTOT, F], F32, addr_space="Shared",
                               name="lc_table")
            h_local = [dr.tile([NPC, F], F32, name=f"h{i}_local")
                       for i in (1, 2, 3)]
            h_table = [dr.tile([NTOT, F], F32, addr_space="Shared",
                               name=f"h{i}_table") for i in (1, 2, 3)]
            pp_local = dr.tile([F, NG], F32, name="pp_local")
            pp_full = dr.tile([F, NG], F32, addr_space="Shared", name="pp_full")

            # library prefetch (mlp lib holds DMAGatherAnt)
            didx = sb.tile([P, 8], I16, name="didx")
            nc.sync.dma_start(out=didx[:, :], in_=dummy_idx[:, :])
            dg = sb.tile([P, F], F32, name="dg")
            nc.gpsimd.dma_gather(
                out_ap=dg[:].rearrange("p (n w) -> p n w", w=F),
                in_ap=dummy_tab[:, :], idxs_ap=didx[:, :],
                num_idxs=P, num_idxs_reg=P, elem_size=F,
                single_packet=False)

            # ---- statics ----
            ident = sb.tile([P, P], F32, name="ident")
            nc.sync.dma_start(out=ident[:, :], in_=ident_in[:, :])
            w1ac_sb = sb.tile([DIN, F], F32, name="w1ac_sb")
            nc.sync.dma_start(out=w1ac_sb[:, :], in_=W1ac[:, :])
            w1bc_sb = sb.tile([DIN, DIN], F32, name="w1bc_sb")
            nc.sync.dma_start(out=w1bc_sb[:, :], in_=W1bc[:, :])
            w2_sb = sb.tile([F, F], F32, name="w2_sb")
            nc.sync.dma_start(out=w2_sb[:, :], in_=W2[:, :])
            w3_sb = sb.tile([F, F], F32, name="w3_sb")
            nc.sync.dma_start(out=w3_sb[:, :], in_=W3[:, :])
            wout_sb = sb.tile([F, DOUT], F32, name="wout_sb")
            nc.sync.dma_start(out=wout_sb[:, :], in_=Wout[:, :])
            b_sb = []
            for nm, t in (("b1r", b1r), ("b2r", b2r), ("b3r", b3r)):
                bb = sb.tile([P, F], F32, name=f"{nm}_sb")
                nc.sync.dma_start(out=bb[:, :], in_=t[:, :])
                b_sb.append(bb)
            boutr_sb = sb.tile([P, DOUT], F32, name="boutr_sb")
            nc.sync.dma_start(out=boutr_sb[:, :], in_=boutr[:, :])
            grid_sb = sb.tile([P, NG], F32, name="grid_sb")
            nc.sync.dma_start(out=grid_sb[:, :], in_=grid[:, :])
            inv3n_sb = sb.tile([P, 4], F32, name="inv3n_sb")
            nc.sync.dma_start(out=inv3n_sb[:, :],
                              in_=inv3n[:].rearrange("(c p) o -> p (c o)", p=P))
            disL_sb = sb.tile([P, TPC], F32, name="disL_sb")
            nc.sync.dma_start(out=disL_sb[:, :],
                              in_=disL_d[:].rearrange("(b p) o -> p (b o)", p=P))
            disA_sb = sb.tile([P, TPC], F32, name="disA_sb")
            nc.sync.dma_start(out=disA_sb[:, :],
                              in_=disA_d[:].rearrange("(b p) o -> p (b o)", p=P))
            batch_sb = sb.tile([P, TPC], F32, name="batch_sb")
            nc.sync.dma_start(out=batch_sb[:, :],
                              in_=batchA_d[:].rearrange("(b p) o -> p (b o)",
                                                        p=P))

            # persistent accumulators
            h1acc = sb.tile([P, TPC * F], F32, name="h1acc")
            x1_all = sb.tile([P, TPC * F], F32, name="x1_all")
            x2_all = sb.tile([P, TPC * F], F32, name="x2_all")
            hs2_all = sb.tile([P, TPC * F], F32, name="hs2_all")
            hs3_all = sb.tile([P, TPC * F], F32, name="hs3_all")

            # ---- phase 0b: A0 into h1acc ----
            XB = 4
            for t4 in range((TPC + 3) // 4):
                bs = [b for b in range(t4 * 4, min(t4 * 4 + 4, TPC))]
                xt = sb.tile([P, XB * P], F32, tag="xph", bufs=2, name="xto")
                nc.sync.dma_start(
                    out=xt[:, :len(bs) * P],
                    in_=XTOWN[:, bs[0] * P:(bs[-1] + 1) * P])
                for j, b in enumerate(bs):
                    pm = ps.tile([P, F], F32, tag="ps_m", bufs=2, name="pma")
                    nc.tensor.matmul(out=pm[:, :],
                                     lhsT=xt[:, j * P:(j + 1) * P],
                                     rhs=w1ac_sb[:, :], start=True, stop=True)
                    nc.scalar.copy(out=h1acc[:, b * F:(b + 1) * F],
                                   in_=pm[:, :])

            # ---- generic dense-stream stage runner ----
            def run_stage(chunks, idx_d, sel_d, table, W, epi,
                          gdt=F32, after_chunk=None):
                idx_off = 0
                sel_off = 0
                for ci, ch in enumerate(chunks):
                    nlo, nhi = ch["nlo"], ch["nhi"]
                    ncols = nlo + nhi
                    npieces = ch["npieces"]
                    idx_sb = sb.tile([P, max(ncols, 1) * 8], I16,
                                     tag="idx", bufs=2, name="idx")
                    if ncols:
                        nc.sync.dma_start(
                            out=idx_sb[:, :ncols * 8],
                            in_=idx_d[:, idx_off * 8:(idx_off + ncols) * 8])
                    g_lo = g_hi = None
                    if nlo:
                        g_lo = sb.tile([P, nlo * W], gdt, tag="glo", bufs=2,
                                       name="glo")
                        nc.gpsimd.dma_gather(
                            out_ap=g_lo[:, :nlo * W]
                            .rearrange("p (n w) -> p n w", w=W),
                            in_ap=table[0:VIEW, :],
                            idxs_ap=idx_sb[:, :nlo * 8],
                            num_idxs=nlo * P, num_idxs_reg=nlo * P,
                            elem_size=W, single_packet=False)
                    if nhi:
                        g_hi = sb.tile([P, nhi * W], gdt, tag="ghi", bufs=2,
                                       name="ghi")
                        nc.gpsimd.dma_gather(
                            out_ap=g_hi[:, :nhi * W]
                            .rearrange("p (n w) -> p n w", w=W),
                            in_ap=table[HI_BASE:NTOT, :],
                            idxs_ap=idx_sb[:, nlo * 8:ncols * 8],
                            num_idxs=nhi * P, num_idxs_reg=nhi * P,
                            elem_size=W, single_packet=False)
                    sel_sb = sb.tile([P, max(npieces, 1) * WIN], gdt,
                                     tag="sel", bufs=2, name="sel")
                    if npieces:
                        nc.sync.dma_start(
                            out=sel_sb[:, :npieces * WIN],
                            in_=sel_d[:, sel_off * WIN:
                                      (sel_off + npieces) * WIN])
                    pi = 0
                    red = {}
                    for evv in ch["events"]:
                        if evv[0] == "e":
                            t_loc = evv[1]
                            epi(t_loc + ci * TPCH, red.pop(t_loc, None), W)
                            continue
                        _, st, col, t_loc, b, st_f, sp_f = evv
                        if t_loc not in red:
                            red[t_loc] = ps.tile([P, W], F32, tag="red",
                                                 bufs=2, name="red")
                        g = g_lo if st == 0 else g_hi
                        nc.tensor.matmul(
                            out=red[t_loc][b * WIN:(b + 1) * WIN, :],
                            lhsT=sel_sb[:, pi * WIN:(pi + 1) * WIN],
                            rhs=g[:, col * W:(col + 1) * W],
                            start=st_f, stop=sp_f,
                            tile_position=(0, b * WIN))
                        pi += 1
                    idx_off += ncols
                    sel_off += npieces
                    if after_chunk is not None:
                        after_chunk(ci)

            # ---- stage epilogues ----
            def epi_spmm1(t, red, W):
                t1s = sb.tile([P, DIN], F32, tag="t1s", bufs=2, name="t1s")
                nc.scalar.copy(out=t1s[:, :], in_=red[:, :])
                tp2 = ps.tile([P, P], F32, tag="ps_t", bufs=1, name="tp2")
                nc.tensor.transpose(out=tp2[:, :], in_=t1s[:, :],
                                    identity=ident[:])
                t1t = sb.tile([P, P], F32, tag="t1t", bufs=2, name="t1t")
                nc.vector.tensor_copy(out=t1t[:, :], in_=tp2[:, :])
                bc_ps = ps.tile([P, DIN], F32, tag="ps_m", bufs=2, name="bc_ps")
                nc.tensor.matmul(out=bc_ps[:, :], lhsT=t1t[:, :],
                                 rhs=w1bc_sb[:, :], start=True, stop=True)
                nc.vector.tensor_add(out=h1acc[:, t * F:(t + 1) * F],
                                     in0=h1acc[:, t * F:(t + 1) * F],
                                     in1=bc_ps[:, 0:F])
                lcs = sb.tile([P, F], F32, tag="lcs", bufs=3, name="lcs")
                nc.scalar.copy(out=lcs[:, :], in_=bc_ps[:, F:DIN])
                nc.sync.dma_start(out=lc_local[t * P:(t + 1) * P, :],
                                  in_=lcs[:, :])

            def epi_spmm2(t, red, W):
                hs = sb.tile([P, F], F32, tag="hs", bufs=3, name="h1s")
                nc.vector.scalar_tensor_tensor(
                    out=hs[:, :], in0=red[:, :], scalar=2.0,
                    in1=h1acc[:, t * F:(t + 1) * F],
                    op0=mybir.AluOpType.mult, op1=mybir.AluOpType.add)
                nc.vector.scalar_tensor_tensor(
                    out=hs[:, :], in0=hs[:, :], scalar=disL_sb[:, t:t + 1],
                    in1=hs[:, :], op0=mybir.AluOpType.mult,
                    op1=mybir.AluOpType.bypass)
                nc.sync.dma_start(out=h_local[0][t * P:(t + 1) * P, :],
                                  in_=hs[:, :])

            pool_ps = ps.tile([F, NG], F32, tag="ps_pool", bufs=1,
                              name="pool_ps")
            pool_n = [0]

            def emit_pool_mm(x_tile_ap, ind_ap):
                i = pool_n[0]
                nc.tensor.matmul(out=pool_ps[:, :], lhsT=x_tile_ap, rhs=ind_ap,
                                 start=(i == 0), stop=(i == 3 * TPC - 1))
                pool_n[0] = i + 1

            def epi_conv(layer, t, red, W):
                xpre = sb.tile([P, F], F32, tag="xpre", bufs=3, name="xpre")
                if layer == 1:
                    nc.vector.scalar_tensor_tensor(
                        out=xpre[:, :], in0=red[:, :],
                        scalar=disA_sb[:, t:t + 1], in1=b_sb[0][:, :],
                        op0=mybir.AluOpType.mult, op1=mybir.AluOpType.add)
                else:
                    hsx = hs2_all if layer == 2 else hs3_all
                    tmp = sb.tile([P, F], F32, tag="tmp", bufs=3, name="tmp")
                    nc.vector.tensor_add(out=tmp[:, :], in0=red[:, :],
                                         in1=hsx[:, t * F:(t + 1) * F])
                    nc.vector.scalar_tensor_tensor(
                        out=xpre[:, :], in0=tmp[:, :],
                        scalar=disA_sb[:, t:t + 1], in1=b_sb[layer - 1][:, :],
                        op0=mybir.AluOpType.mult, op1=mybir.AluOpType.add)
                if layer == 1:
                    xt_ap = x1_all[:, t * F:(t + 1) * F]
                elif layer == 2:
                    xt_ap = x2_all[:, t * F:(t + 1) * F]
                else:
                    x3t = sb.tile([P, F], F32, tag="x3t", bufs=3, name="x3t")
                    xt_ap = x3t[:, :]
                nc.scalar.activation(out=xt_ap, in_=xpre[:, :],
                                     func=mybir.ActivationFunctionType.Relu)
                if layer < 3:
                    xs = sb.tile([P, F], F32, tag="xs", bufs=3, name="xs")
                    nc.vector.scalar_tensor_tensor(
                        out=xs[:, :], in0=xt_ap, scalar=disA_sb[:, t:t + 1],
                        in1=xt_ap, op0=mybir.AluOpType.mult,
                        op1=mybir.AluOpType.bypass)
                    tp = ps.tile([F, P], F32, tag="ps_t", bufs=1, name="tp")
                    nc.tensor.transpose(out=tp[:, :], in_=xs[:, :],
                                        identity=ident[:])
                    xsT = sb.tile([F, P], F32, tag="xsT", bufs=3, name="xsT")
                    nc.vector.tensor_copy(out=xsT[:, :], in_=tp[:, :])
                    hm = ps.tile([P, F], F32, tag="ps_m", bufs=2, name="hm")
                    wnext = w2_sb if layer == 1 else w3_sb
                    nc.tensor.matmul(out=hm[:, :], lhsT=xsT[:, :],
                                     rhs=wnext[:, :], start=True, stop=True)
                    hsx = hs2_all if layer == 1 else hs3_all
                    nc.scalar.copy(out=hsx[:, t * F:(t + 1) * F], in_=hm[:, :])
                    nc.sync.dma_start(out=h_local[layer][t * P:(t + 1) * P, :],
                                      in_=hsx[:, t * F:(t + 1) * F])
                else:
                    ind = sb.tile([P, NG], F32, tag="ind", bufs=2, name="ind")
                    nc.vector.tensor_tensor(
                        out=ind[:, :],
                        in0=batch_sb[:, t:t + 1].to_broadcast([P, NG]),
                        in1=grid_sb[:, :], op=mybir.AluOpType.is_equal)
                    emit_pool_mm(x1_all[:, t * F:(t + 1) * F], ind[:, :])
                    emit_pool_mm(x2_all[:, t * F:(t + 1) * F], ind[:, :])
                    emit_pool_mm(xt_ap, ind[:, :])

            def ag(local, table_t):
                nc.gpsimd.collective_compute(
                    "AllGather", mybir.AluOpType.bypass,
                    replica_groups=[list(range(NCORES))],
                    ins=[local[:, :]], outs=[table_t[:, :]])

            # ---- run the 5 stages ----
            run_stage(chunksL, IDXL, SELL, XP, DIN, epi_spmm1, gdt=BF16)
            ag(lc_local, lc_table)
            run_stage(chunksL, IDXL, SELL2, lc_table, F, epi_spmm2)
            ag(h_local[0], h_table[0])
            run_stage(chunksA1, IDXA1, SELA1, h_table[0], F,
                      lambda t, r, W: epi_conv(1, t, r, W))
            ag(h_local[1], h_table[1])
            run_stage(chunksA23, IDXA23, SELA23, h_table[1], F,
                      lambda t, r, W: epi_conv(2, t, r, W))
            ag(h_local[2], h_table[2])
            run_stage(chunksA23, IDXA23, SELA23, h_table[2], F,
                      lambda t, r, W: epi_conv(3, t, r, W))

            # ---- pool partials -> AllReduce -> head ----
            pool_sb = sb.tile([F, NG], F32, name="pool_sb")
            nc.vector.tensor_copy(out=pool_sb[:, :], in_=pool_ps[:, :])
            nc.sync.dma_start(out=pp_local[:, :], in_=pool_sb[:, :])
            nc.gpsimd.collective_compute(
                "AllReduce", mybir.AluOpType.add,
                replica_groups=[list(range(NCORES))],
                ins=[pp_local[:, :]], outs=[pp_full[:, :]])

            pp_sb = sb.tile([F, NG], F32, name="pp_sb")
            nc.sync.dma_start(out=pp_sb[:, :], in_=pp_full[:, :])
            zt_ps = ps.tile([DOUT, NG], F32, tag="ps_z", bufs=1, name="zt_ps")
            nc.tensor.matmul(out=zt_ps[:, :], lhsT=wout_sb[:, :],
                             rhs=pp_sb[:, :], start=True, stop=True)
            zt_sb = sb.tile([DOUT, NG], F32, name="zt_sb")
            nc.vector.tensor_copy(out=zt_sb[:, :], in_=zt_ps[:, :])
            for c4 in range(4):
                tr = ps.tile([P, DOUT], F32, tag="ps_t2", bufs=1, name="tr")
                nc.tensor.transpose(out=tr[:, :],
                                    in_=zt_sb[:, c4 * P:(c4 + 1) * P],
                                    identity=ident[:DOUT, :DOUT])
                y = sb.tile([P, DOUT], F32, tag="ysm", bufs=2, name="y")
                nc.vector.scalar_tensor_tensor(
                    out=y[:, :], in0=tr[:, :], scalar=inv3n_sb[:, c4:c4 + 1],
                    in1=boutr_sb[:, :],
                    op0=mybir.AluOpType.mult, op1=mybir.AluOpType.add)
                mx = sb.tile([P, 1], F32, tag="mx", bufs=2, name="mx")
                nc.vector.tensor_reduce(out=mx[:, :], in_=y[:, :],
                                        axis=mybir.AxisListType.X,
                                        op=mybir.AluOpType.max)
                nmx = sb.tile([P, 1], F32, tag="nmx", bufs=2, name="nmx")
                nc.vector.tensor_scalar_mul(out=nmx[:, :], in0=mx[:, :],
                                            scalar1=-1.0)
                ex = sb.tile([P, DOUT], F32, tag="ex", bufs=2, name="ex")
                ssum = sb.tile([P, 1], F32, tag="ssum", bufs=2, name="ssum")
                nc.scalar.activation(out=ex[:, :], in_=y[:, :],
                                     func=mybir.ActivationFunctionType.Exp,
                                     bias=nmx[:, :1], scale=1.0,
                                     accum_out=ssum[:, :1])
                rs = sb.tile([P, 1], F32, tag="rs", bufs=2, name="rs")
                nc.vector.reciprocal(out=rs[:, :], in_=ssum[:, :])
                ot = sb.tile([P, DOUT], F32, tag="ot", bufs=2, name="ot")
                nc.vector.tensor_scalar_mul(out=ot[:, :], in0=ex[:, :],
                                            scalar1=rs[:, :1])
                nc.sync.dma_start(out=OUT[c4 * P:(c4 + 1) * P, :],
                                  in_=ot[:, :])

    nc.compile()
    return nc


# ---------------- public entry ----------------

def kernel(X, L_indices, L_values, batch,
           W1, b1, W2, b2, W3, b3, Wout, bout):
    global LAST_EXEC_TIME_NS
    assert X.shape == (N, DIN)
    in_maps, meta = _prep(np.asarray(X), np.asarray(L_indices),
                          np.asarray(L_values), np.asarray(batch),
                          np.asarray(W1), np.asarray(W2), np.asarray(W3),
                          np.asarray(Wout), np.asarray(b1), np.asarray(b2),
                          np.asarray(b3), np.asarray(bout))
    nc = _build_program(meta)
    res = run_bass_kernel_spmd(nc, in_maps, core_ids=list(range(NCORES)))
    LAST_EXEC_TIME_NS = res.exec_time_ns
    if res.exec_time_ns is not None:
        print(f"HW exec time: {res.exec_time_ns} ns")
    return res.results[0]["out"]
